# revision 4
# baseline (speedup 1.0000x reference)
# NNUE embedding-bag kernel for 8 Trainium2 NeuronCores.
# Strategy: data-parallel over batch (2048 bags/core). Per 128-bag tile:
# build exact per-bag feature-count vectors with GPSIMD local_scatter
# (scattering prefix-duplicate-counts so the last write holds the total),
# PE-transpose them to feature-major, matmul against the bf16 table,
# fused bias+relu on ACT plus min-clip on DVE, then a small per-tile head
# matmul with the head bias folded in as an extra contraction row and a
# window-compare bucket mask to select one of the 8 scores.
import os
import sys

import numpy as np

for _p in ("/opt/trn_rl_repo", "/root/.axon_site/_ro/trn_rl_repo"):
    if os.path.isdir(_p) and _p not in sys.path:
        sys.path.insert(0, _p)

import ml_dtypes

B, BAG, L1, NF = 16384, 32, 512, 768  # NF: real features; index 768 is PAD
NCORES = 8
BS = B // NCORES        # bags per core
NT = BS // 128          # 16 batch tiles of 128 bags
NST = NT // 4           # 4 supertiles of 512 bags
NE = 770                # local_scatter num_elems (>=769, even)
FC = NF // 128          # 6 feature chunks
LC = L1 // 128          # 4 l1 chunks

_cache = {}
last_results = None


def _build():
    import concourse.bass as bass
    import concourse.mybir as mybir
    from concourse import bacc, library_config
    from concourse.tile import TileContext

    dt = mybir.dt
    alu = mybir.AluOpType
    act = mybir.ActivationFunctionType

    nc = bacc.Bacc("TRN2", target_bir_lowering=False, debug=False)

    stm_d = nc.dram_tensor("stm", [BS, BAG], dt.int16, kind="ExternalInput")
    nstm_d = nc.dram_tensor("nstm", [BS, BAG], dt.int16, kind="ExternalInput")
    tbl_d = nc.dram_tensor("tbl", [NF, L1], dt.bfloat16, kind="ExternalInput")
    bias_d = nc.dram_tensor("bias", [128, 2 * LC], dt.float32, kind="ExternalInput")
    hwt_d = nc.dram_tensor("hwt", [128, 64], dt.bfloat16, kind="ExternalInput")
    hb_d = nc.dram_tensor("hb", [1, 8], dt.float32, kind="ExternalInput")
    ident_d = nc.dram_tensor("ident", [128, 128], dt.bfloat16, kind="ExternalInput")
    iota9_d = nc.dram_tensor("iota9", [128, 9], dt.float32, kind="ExternalInput")
    ones128_d = nc.dram_tensor("ones128", [1, 128], dt.float32, kind="ExternalInput")
    out_d = nc.dram_tensor("out", [BS], dt.float32, kind="ExternalOutput")

    with TileContext(nc) as tc:
        with (
            tc.tile_pool(name="consts", bufs=1) as cpool,
            tc.tile_pool(name="idx", bufs=2) as ipool,
            tc.tile_pool(name="work", bufs=3) as wpool,
            tc.tile_pool(name="hist", bufs=3) as hpool,
            tc.tile_pool(name="hT", bufs=2) as htpool,
            tc.tile_pool(name="emb", bufs=2) as epool,
            tc.tile_pool(name="small", bufs=3) as spool,
            tc.tile_pool(name="tr_ps", bufs=2, space="PSUM") as trppool,
            tc.tile_pool(name="mm_ps", bufs=2, space="PSUM") as mmppool,
            tc.tile_pool(name="hd_ps", bufs=2, space="PSUM") as hdppool,
        ):
            nc.gpsimd.load_library(library_config.local_scatter)

            t_sb = cpool.tile([128, FC, L1], dt.bfloat16)
            nc.sync.dma_start(
                out=t_sb, in_=tbl_d.ap().rearrange("(c p) l -> p c l", p=128)
            )
            bias_sb = cpool.tile([128, 2 * LC], dt.float32)
            nc.sync.dma_start(out=bias_sb, in_=bias_d.ap())
            hwt_sb = cpool.tile([128, 8, 8], dt.bfloat16)
            nc.sync.dma_start(
                out=hwt_sb, in_=hwt_d.ap().rearrange("p (c h) -> p c h", h=8)
            )
            hb_sb = cpool.tile([1, 8], dt.float32)
            nc.sync.dma_start(out=hb_sb, in_=hb_d.ap())
            ident_sb = cpool.tile([128, 128], dt.bfloat16)
            nc.sync.dma_start(out=ident_sb, in_=ident_d.ap())
            iota9_sb = cpool.tile([128, 9], dt.float32)
            nc.sync.dma_start(out=iota9_sb, in_=iota9_d.ap())
            ones128_sb = cpool.tile([1, 128], dt.float32)
            nc.sync.dma_start(out=ones128_sb, in_=ones128_d.ap())
            out_sb = cpool.tile([128, NT], dt.float32)

            idx_all = {}
            for side, src in (("stm", stm_d), ("nstm", nstm_d)):
                it = ipool.tile([128, NT, BAG], dt.int16, tag=f"idx_{side}")
                nc.sync.dma_start(
                    out=it, in_=src.ap().rearrange("(p t) j -> p t j", t=NT)
                )
                idx_all[side] = it

            for st in range(NST):
                mask_st = spool.tile([128, 4, 8], dt.bfloat16, tag="mask_st")
                embt = epool.tile([128, 2 * LC, 512], dt.bfloat16, tag="embt")
                for si, side in enumerate(("stm", "nstm")):
                    ht = htpool.tile([128, FC, 512], dt.bfloat16, tag="ht")
                    for bt in range(4):
                        t = st * 4 + bt
                        # padded index tile: [0:32) sentinel -1, [32:64) idx
                        ipad = wpool.tile([128, 2 * BAG], dt.int16, tag="ipad")
                        nc.vector.memset(ipad[:, 0:BAG], -1)
                        nc.vector.tensor_copy(
                            ipad[:, BAG : 2 * BAG], idx_all[side][:, t, :]
                        )
                        # eq[p, j, o] = (idx[p, j] == ipad[p, 1 + j + o])
                        # (slots j-31..j-1 in shifted coords; sentinel kills
                        # out-of-bag lanes)
                        eq = wpool.tile([128, BAG, BAG - 1], dt.bfloat16, tag="eq")
                        in0 = (
                            ipad[:, BAG : 2 * BAG]
                            .unsqueeze(2)
                            .broadcast_to([128, BAG, BAG - 1])
                        )
                        in1 = bass.AP(
                            ipad.tensor,
                            ipad.offset + 1,
                            [list(ipad.ap[0]), [1, BAG], [1, BAG - 1]],
                        )
                        nc.vector.tensor_tensor(eq, in0, in1, op=alu.is_equal)
                        cnt_e = wpool.tile([128, BAG], dt.float32, tag="cnt_e")
                        nc.vector.tensor_reduce(
                            cnt_e, eq, axis=mybir.AxisListType.X, op=alu.add
                        )
                        cntb = wpool.tile([128, BAG], dt.bfloat16, tag="cntb")
                        nc.vector.tensor_scalar(
                            out=cntb, in0=cnt_e, scalar1=1.0, scalar2=None,
                            op0=alu.add,
                        )
                        h = hpool.tile([128, NE], dt.bfloat16, tag="h")
                        nc.gpsimd.local_scatter(
                            h, cntb, ipad[:, BAG : 2 * BAG],
                            channels=128, num_elems=NE, num_idxs=BAG,
                        )
                        for fc in range(FC):
                            trp = trppool.tile([128, 128], dt.bfloat16)
                            nc.tensor.transpose(
                                trp, h[:, fc * 128 : (fc + 1) * 128], ident_sb
                            )
                            nc.scalar.copy(
                                ht[:, fc, bt * 128 : (bt + 1) * 128], trp
                            )
                        if si == 0:
                            # bucket mask from stm indices
                            junk = spool.tile([128, BAG], dt.bfloat16, tag="junk")
                            cntp = spool.tile([128, 1], dt.float32, tag="cntp")
                            nc.vector.tensor_scalar(
                                out=junk, in0=ipad[:, BAG : 2 * BAG],
                                scalar1=768.0, scalar2=0.0, op0=alu.is_equal,
                                op1=alu.add, accum_out=cntp,
                            )
                            v = spool.tile([128, 1], dt.float32, tag="v")
                            nc.vector.tensor_scalar(
                                out=v, in0=cntp, scalar1=-0.25, scalar2=7.5,
                                op0=alu.mult, op1=alu.add,
                            )
                            ge9 = spool.tile([128, 9], dt.float32, tag="ge9")
                            nc.vector.tensor_scalar(
                                out=ge9, in0=iota9_sb, scalar1=v,
                                scalar2=None, op0=alu.is_le,
                            )
                            nc.vector.tensor_tensor(
                                mask_st[:, bt, :], ge9[:, 0:8], ge9[:, 1:9],
                                op=alu.subtract,
                            )
                    for lc in range(LC):
                        mmp = mmppool.tile([128, 512], dt.float32)
                        for fc in range(FC):
                            nc.tensor.matmul(
                                mmp,
                                t_sb[:, fc, lc * 128 : (lc + 1) * 128],
                                ht[:, fc, :],
                                start=(fc == 0),
                                stop=(fc == FC - 1),
                            )
                        c = si * LC + lc
                        nc.scalar.activation(
                            embt[:, c, :], mmp, act.Relu,
                            bias=bias_sb[:, c : c + 1],
                        )
                        nc.vector.tensor_scalar(
                            out=embt[:, c, :], in0=embt[:, c, :],
                            scalar1=1.0, scalar2=None, op0=alu.min,
                        )
                for bt in range(4):
                    t = st * 4 + bt
                    hdp = hdppool.tile([128, 8], dt.float32)
                    for c in range(2 * LC):
                        nc.tensor.matmul(
                            hdp,
                            embt[:, c, bt * 128 : (bt + 1) * 128],
                            hwt_sb[:, c, :],
                            start=(c == 0),
                            stop=False,
                        )
                    nc.tensor.matmul(
                        hdp, ones128_sb, hb_sb, start=False, stop=True,
                    )
                    junk8 = spool.tile([128, 8], dt.float32, tag="junk8")
                    nc.vector.scalar_tensor_tensor(
                        out=junk8, in0=mask_st[:, bt, :], scalar=1.0,
                        in1=hdp, op0=alu.mult, op1=alu.mult,
                        accum_out=out_sb[:, t : t + 1],
                    )
            nc.sync.dma_start(
                out=out_d.ap().rearrange("(p t) -> p t", t=NT), in_=out_sb
            )
    nc.compile()
    return nc


def kernel(stm_indices, nstm_indices, emb_table, emb_bias, head_w, head_b):
    global last_results
    from concourse.bass_utils import run_bass_kernel_spmd

    if "nc" not in _cache:
        _cache["nc"] = _build()
    nc = _cache["nc"]

    stm = np.asarray(stm_indices).astype(np.int16)
    nstm = np.asarray(nstm_indices).astype(np.int16)
    tbl = np.asarray(emb_table, dtype=np.float32)[:NF].astype(ml_dtypes.bfloat16)
    bias1024 = np.concatenate(
        [np.asarray(emb_bias, np.float32)] * 2
    ).reshape(2 * LC, 128).T.copy()  # [128, 8]
    hw = np.asarray(head_w, dtype=np.float32)  # [8, 1024]
    hwt = hw.reshape(8, 8, 128).transpose(2, 1, 0).reshape(128, 64)
    hwt = hwt.astype(ml_dtypes.bfloat16)
    hb = np.asarray(head_b, np.float32).reshape(1, 8)
    ident = np.eye(128, dtype=ml_dtypes.bfloat16)
    iota9 = np.tile(
        np.array([-100, 1, 2, 3, 4, 5, 6, 7, 8], np.float32), (128, 1)
    )
    ones128 = np.ones((1, 128), np.float32)

    in_maps = []
    for c in range(NCORES):
        sl = slice(c * BS, (c + 1) * BS)
        in_maps.append({
            "stm": np.ascontiguousarray(stm[sl]),
            "nstm": np.ascontiguousarray(nstm[sl]),
            "tbl": tbl, "bias": bias1024, "hwt": hwt, "hb": hb,
            "ident": ident, "iota9": iota9, "ones128": ones128,
        })
    trace = os.environ.get("BASS_KERNEL_TRACE", "0") == "1"
    res = run_bass_kernel_spmd(
        nc, in_maps, core_ids=list(range(NCORES)), trace=trace
    )
    last_results = res
    out = np.concatenate([res.results[c]["out"] for c in range(NCORES)])
    return out.reshape(B, 1).astype(np.float32)
